# revision 21
# baseline (speedup 1.0000x reference)
"""Trainium2 Bass kernel: fused speculative-decoding rejection sampling.

Problem: B=64 requests x K=8 draft tokens over V=128000 vocab.
Data-parallel over 8 NeuronCores (8 requests per core). Each core:
  1. indirect-gathers p=target[b,k,d], q=draft[b,k,d] (64 scalars, one per
     partition -- HW indirect DMA consumes one index per output partition),
  2. computes the accept/reject chain with small block-triangular matmuls
     -> num_accepted, first-reject position j, u_j, all in [8,1] layout,
  3. indirect-gathers ONLY the j-th rows of target/draft probs
     (2 tensors x 2 blocks x 2 halves x 1MB; row layout: 32 partitions x
     4000) -- the memory-roofline trick: 8MB/core instead of 64MB/core,
  4. residual r = max(t-d,0) via min + fused scan (state=(t+state)-min,
     chained across halves), partition-prefix via block-triangular matmul,
     recovered index = #{cdf <= u*total} counted with Sign/Square on the
     Scalar engine (exact: a1=sum sign(c-R), a2=sum sign^2 ->
     count = N - (a1+a2)/2; ties give sign 0 and cancel),
  5. assembles output tokens + counts on device.
All tiny per-core scalars (token ids, uniforms, flat gather indices, bonus)
ride along in the konst tensor as exact f32 (< 2^24). The host only shards
inputs, packs konst, unpacks the device outputs, and handles the degenerate
total<=0 fallback (argmax row) which cannot occur for normalized softmax
inputs but is matched for exactness.
"""

import numpy as np

import concourse.bacc as bacc
import concourse.mybir as mybir
from concourse.bass import IndirectOffsetOnAxis
from concourse.bass_utils import run_bass_kernel_spmd
from concourse.tile import TileContext

B, K, V = 64, 8, 128000
NCORES = 8
BL = B // NCORES          # 8 batch rows per core
BK = BL * K               # 64 (b,k) pairs per core
P = 128                   # SBUF partitions
RP = 32                   # partitions per gathered row
CH = V // RP              # 4000 row elements per partition
HCH = CH // 2             # half-chunk (per-gather free extent)
NBLK = 2                  # batch blocks (4 batches each)
BPB = BL // NBLK          # 4 batches per block
PLACEHOLDER = -1

# konst (f32 [128, KW]) column layout
C_IOTA32 = 0              # [128,1] p % 32
C_KP1 = 1                 # [64,1] m%8 + 1
C_K64 = 2                 # [64,1] m%8
C_C0 = 3                  # [64,1] 1 if m%8==0
C_ROWB = 4                # [8,1] 8*b
C_POS = 5                 # [8, 5:14] = 0..8
C_LINC = 16               # [64, 16:80]  (k//8==m//8)&(k<=m)      lhsT[k,m]
C_SH = 80                 # [64, 80:144] first-reject shift matrix lhsT[k,m]
C_GEXT = 144              # [64, 144:152] (m//8 == b)             lhsT[m,b]
C_SELB1 = 152             # [8, 152:280]  (p//32 == b%4)          lhsT[b,p]
C_SELB32 = 280            # [8, 280:408]  32*(p//32 == b%4)       lhsT[b,p]
C_PL32 = 408              # [128, 408:536] (k//32==m//32)&(k<m)   lhsT[k,m]
C_G32A = 536              # [128, 536:544] (k//32==b%4)&(b<4)     lhsT[k,b]
C_G32B = 544              # [128, 544:552] (k//32==b%4)&(b>=4)    lhsT[k,b]
C_AUXFI = 552             # [64,1] flat gather index (b*K+k)*V + d (exact f32)
C_AUXU = 553              # [64,1] uniform samples, (b,k) on partitions
C_AUXD = 554              # [8, 554:563] draft tokens padded with a zero col
C_AUXB = 563              # [8,1] bonus token ids
C_BLKM = 564              # [8, 564:566] 1 if b//4 == blk
C_SELBN1 = 566            # [8, 566:694]  -(p//32 == b%4)         lhsT[b,p]
C_NL32 = 694              # [128, 694:822] -((k//32==m//32)&(k<m)) lhsT[k,m]
KW = 822

AT = mybir.AluOpType
AF = mybir.ActivationFunctionType
F32 = mybir.dt.float32
I32 = mybir.dt.int32


def _build_konst(dloc: np.ndarray, uloc: np.ndarray, bloc: np.ndarray
                 ) -> np.ndarray:
    """dloc [BL,K] int draft ids, uloc [BL,K] f32 uniforms, bloc [BL] bonus."""
    k = np.zeros((P, KW), np.float32)
    p = np.arange(P)
    m = np.arange(BK)
    b8 = np.arange(BL)
    k[:, C_IOTA32] = 2 * (p % RP)   # half-row units
    k[:BK, C_KP1] = m % K + 1
    k[:BK, C_K64] = m % K
    k[:BK, C_C0] = (m % K == 0)
    k[:BL, C_ROWB] = b8 * K
    k[:BL, C_POS:C_POS + 9] = np.arange(K + 1)[None, :]
    k[:BK, C_LINC:C_LINC + BK] = (
        (m[:, None] // K == m[None, :] // K) & (m[:, None] <= m[None, :])
    )
    # oh_m = SH^T@alive + c0: first-reject one-hot (j = min(num_acc, K-1))
    sh = np.zeros((BK, BK), np.float32)
    for mm in range(BK):
        r = mm % K
        if r == 0:
            sh[mm, mm] = -1.0            # oh_0 = 1 - alive_0 (c0 adds the 1)
        elif r < K - 1:
            sh[mm - 1, mm] = 1.0         # oh_m = alive_{m-1} - alive_m
            sh[mm, mm] = -1.0
        else:
            sh[mm - 1, mm] = 1.0         # oh_7 = alive_6
    k[:BK, C_SH:C_SH + BK] = sh
    k[:BK, C_GEXT:C_GEXT + BL] = (m[:, None] // K == b8[None, :])
    selb = (p[None, :] // RP == (b8 % BPB)[:, None]).astype(np.float32)
    k[:BL, C_SELB1:C_SELB1 + P] = selb
    k[:BL, C_SELB32:C_SELB32 + P] = 2 * RP * selb   # half-row units
    k[:, C_PL32:C_PL32 + P] = (
        (p[:, None] // RP == p[None, :] // RP) & (p[:, None] < p[None, :])
    )
    ga = (p[:, None] // RP == (b8 % BPB)[None, :]).astype(np.float32)
    k[:, C_G32A:C_G32A + BL] = ga * (b8[None, :] // BPB == 0)
    k[:, C_G32B:C_G32B + BL] = ga * (b8[None, :] // BPB == 1)
    k[:BK, C_AUXFI] = (m * V + dloc.reshape(BK)).astype(np.float32)
    k[:BK, C_AUXU] = uloc.reshape(BK)
    k[:BL, C_AUXD:C_AUXD + K] = dloc
    k[:BL, C_AUXB] = bloc
    k[:BL, C_BLKM:C_BLKM + NBLK] = (b8[:, None] // BPB == np.arange(NBLK)[None, :])
    k[:BL, C_SELBN1:C_SELBN1 + P] = -selb
    k[:, C_NL32:C_NL32 + P] = -k[:, C_PL32:C_PL32 + P]
    return k


def build_nc():
    nc = bacc.Bacc("TRN2", target_bir_lowering=False, debug=False)

    dpr = nc.declare_dram_parameter("draft_probs", [BL, K, V], F32, isOutput=False)
    tpr = nc.declare_dram_parameter("target_probs", [BL, K, V], F32, isOutput=False)
    kon = nc.declare_dram_parameter("konst", [P, KW], F32, isOutput=False)
    out_i = nc.declare_dram_parameter("out_i", [BL, 16], I32, isOutput=True)
    out_f = nc.declare_dram_parameter("out_f", [BL, 2], F32, isOutput=True)

    # DRAM views for the indirect gathers (offset must be 0)
    tp_flat = tpr.ap().rearrange("b k (v one) -> (b k v) one", one=1)
    dp_flat = dpr.ap().rearrange("b k (v one) -> (b k v) one", one=1)
    # [BL*K*64, 2000] half-row-chunk table (coef = HCH per index step)
    tp_rows = tpr.ap().rearrange("b k (h l) -> (b k h) l", l=HCH)
    dp_rows = dpr.ap().rearrange("b k (h l) -> (b k h) l", l=HCH)

    with TileContext(nc) as tc:
        with (
            tc.tile_pool(name="sb", bufs=1) as sp,
            tc.tile_pool(name="ps", bufs=2, space="PSUM") as pp,
        ):
            kt = sp.tile([P, KW], F32)
            nc.sync.dma_start(out=kt[:], in_=kon[:, :])
            u64 = kt[0:BK, C_AUXU:C_AUXU + 1]

            # ---- p/q element gathers (one element per partition) ----
            fidx_i = sp.tile([BK, 1], I32)
            nc.vector.tensor_copy(out=fidx_i[:],
                                  in_=kt[0:BK, C_AUXFI:C_AUXFI + 1])
            p64 = sp.tile([BK, 1], F32)
            q64 = sp.tile([BK, 1], F32)
            nc.gpsimd.indirect_dma_start(
                out=p64[:], out_offset=None, in_=tp_flat,
                in_offset=IndirectOffsetOnAxis(ap=fidx_i[:], axis=0),
            )
            nc.gpsimd.indirect_dma_start(
                out=q64[:], out_offset=None, in_=dp_flat,
                in_offset=IndirectOffsetOnAxis(ap=fidx_i[:], axis=0),
            )

            # ---- accept chain: accept = u*max(q,1e-10) < p ----
            qc = sp.tile([BK, 1], F32)
            nc.vector.tensor_scalar_max(qc[:], q64[:], 1e-10)
            uq = sp.tile([BK, 1], F32)
            nc.vector.tensor_mul(out=uq[:], in0=u64, in1=qc[:])
            acc = sp.tile([BK, 1], F32)
            nc.vector.tensor_tensor(out=acc[:], in0=uq[:], in1=p64[:], op=AT.is_lt)
            ac_ps = pp.tile([BK, 1], F32, tag="pst")
            nc.tensor.matmul(out=ac_ps[:], lhsT=kt[0:BK, C_LINC:C_LINC + BK],
                             rhs=acc[:])
            rhs3 = sp.tile([BK, 3], F32)
            # alive_m = (#accepts through m == m%8+1)
            nc.vector.tensor_tensor(
                out=rhs3[:, 0:1], in0=ac_ps[:], in1=kt[0:BK, C_KP1:C_KP1 + 1],
                op=AT.is_equal,
            )
            oh_ps = pp.tile([BK, 1], F32, tag="pst")
            nc.tensor.matmul(out=oh_ps[:], lhsT=kt[0:BK, C_SH:C_SH + BK],
                             rhs=rhs3[:, 0:1])
            oh = sp.tile([BK, 1], F32)
            nc.vector.tensor_add(out=oh[:], in0=oh_ps[:],
                                 in1=kt[0:BK, C_C0:C_C0 + 1])
            nc.vector.tensor_mul(out=rhs3[:, 1:2], in0=oh[:],
                                 in1=kt[0:BK, C_K64:C_K64 + 1])
            nc.vector.tensor_mul(out=rhs3[:, 2:3], in0=oh[:], in1=u64)
            nj_ps = pp.tile([BL, 3], F32, tag="pst")
            nc.tensor.matmul(out=nj_ps[:], lhsT=kt[0:BK, C_GEXT:C_GEXT + BL],
                             rhs=rhs3[:])
            nj = sp.tile([BL, 3], F32)
            nc.vector.tensor_copy(out=nj[:], in_=nj_ps[:])
            num_acc = nj[:, 0:1]
            jv = nj[:, 1:2]
            u_j = nj[:, 2:3]

            # ---- block chunk indices: idx[p,blk] = rowidx*32 + p%32 ----
            rowidx = sp.tile([BL, 1], F32)
            nc.vector.tensor_add(out=rowidx[:], in0=jv,
                                 in1=kt[0:BL, C_ROWB:C_ROWB + 1])
            rsplit = sp.tile([BL, NBLK], F32)
            nc.vector.tensor_scalar(
                rsplit[:], kt[0:BL, C_BLKM:C_BLKM + NBLK], rowidx[:], None,
                op0=AT.mult,
            )
            idx_ps = pp.tile([P, NBLK], F32, tag="pst")
            nc.tensor.matmul(out=idx_ps[:], lhsT=kt[0:BL, C_SELB32:C_SELB32 + P],
                             rhs=rsplit[:])
            idx_i = sp.tile([P, NBLK], I32)
            nc.vector.tensor_scalar(
                idx_i[:], idx_ps[:], kt[:, C_IOTA32:C_IOTA32 + 1], None,
                op0=AT.add,
            )

            # ---- gather the selected rows, 1MB per gather ----
            TG = sp.tile([P, NBLK * CH], F32)
            DR = sp.tile([P, NBLK * CH], F32)
            M = sp.tile([P, NBLK * CH], F32)
            C = sp.tile([P, NBLK * CH], F32)
            for blk in range(NBLK):
                for h in range(2):
                    cs = slice(blk * CH + h * HCH, blk * CH + (h + 1) * HCH)
                    nc.gpsimd.indirect_dma_start(
                        out=TG[:, cs], out_offset=None, in_=tp_rows,
                        in_offset=IndirectOffsetOnAxis(
                            ap=idx_i[:, blk:blk + 1], axis=0),
                        element_offset=h * HCH,
                    )
                    nc.gpsimd.indirect_dma_start(
                        out=DR[:, cs], out_offset=None, in_=dp_rows,
                        in_offset=IndirectOffsetOnAxis(
                            ap=idx_i[:, blk:blk + 1], axis=0),
                        element_offset=h * HCH,
                    )

            # ---- per block: residual cumsum, thresholds, counts ----
            asum = sp.tile([P, NBLK], F32)
            a2t = sp.tile([P, NBLK], F32)
            tot_sb = sp.tile([BL, NBLK], F32)
            negr = sp.tile([P, NBLK], F32)
            for blk in range(NBLK):
                for h in range(2):
                    cs = slice(blk * CH + h * HCH, blk * CH + (h + 1) * HCH)
                    nc.vector.tensor_tensor(
                        out=M[:, cs], in0=TG[:, cs], in1=DR[:, cs], op=AT.min)
                    init = 0.0 if h == 0 else C[:, blk * CH + HCH - 1:
                                                blk * CH + HCH]
                    nc.vector.tensor_tensor_scan(
                        out=C[:, cs], data0=TG[:, cs], data1=M[:, cs],
                        initial=init, op0=AT.add, op1=AT.subtract,
                    )
                # matmuls read the scan's last column (the row totals)
                Scol = C[:, (blk + 1) * CH - 1:(blk + 1) * CH]
                # per-block totals (other block's batches read as zero)
                totb_ps = pp.tile([BL, 1], F32, tag="pst")
                GXX = C_G32A if blk == 0 else C_G32B
                nc.tensor.matmul(out=totb_ps[:], lhsT=kt[:, GXX:GXX + BL],
                                 rhs=Scol)
                nc.scalar.copy(out=tot_sb[:, blk:blk + 1], in_=totb_ps[:])
                # thr = u_j * total (Scalar engine; keeps DVE streaming)
                thrb = sp.tile([BL, 1], F32, tag="thrn")
                nc.scalar.activation(out=thrb[:], in_=totb_ps[:],
                                     func=AF.Copy, bias=0.0, scale=u_j)
                col = slice(blk, blk + 1)
                negr_ps = pp.tile([P, 1], F32, tag="pst")
                if blk == 0:
                    # negR[p] = P_pref - thr (bias for the Sign count)
                    nc.tensor.matmul(out=negr_ps[:],
                                     lhsT=kt[0:BL, C_SELBN1:C_SELBN1 + P],
                                     rhs=thrb[:], start=True, stop=False)
                    nc.tensor.matmul(out=negr_ps[:],
                                     lhsT=kt[:, C_PL32:C_PL32 + P],
                                     rhs=Scol, start=False, stop=True)
                else:
                    # posR[p] = thr - P_pref (threshold for the DVE count)
                    nc.tensor.matmul(out=negr_ps[:],
                                     lhsT=kt[0:BL, C_SELB1:C_SELB1 + P],
                                     rhs=thrb[:], start=True, stop=False)
                    nc.tensor.matmul(out=negr_ps[:],
                                     lhsT=kt[:, C_NL32:C_NL32 + P],
                                     rhs=Scol, start=False, stop=True)
                if blk == 0:
                    nc.scalar.copy(out=negr[:, col], in_=negr_ps[:])
                if blk == 0:
                    # exact count on the Scalar engine (overlaps block-1 DVE):
                    # a1+a2 = 2*#{c > R}; ties give sign 0 and cancel
                    for h in range(2):
                        cs = slice(blk * CH + h * HCH,
                                   blk * CH + (h + 1) * HCH)
                        nc.scalar.activation(
                            out=M[:, cs], in_=C[:, cs], func=AF.Sign,
                            bias=negr[:, col], scale=1.0,
                            accum_out=(asum[:, col] if h == 0 else a2t[:, col]),
                        )
                    nc.vector.tensor_add(out=asum[:, col], in0=asum[:, col],
                                         in1=a2t[:, col])
                    for h in range(2):
                        cs = slice(blk * CH + h * HCH,
                                   blk * CH + (h + 1) * HCH)
                        nc.scalar.activation(
                            out=TG[:, cs], in_=M[:, cs], func=AF.Square,
                            accum_out=a2t[:, col],
                        )
                        nc.vector.tensor_add(out=asum[:, col],
                                             in0=asum[:, col],
                                             in1=a2t[:, col])
                else:
                    # last block: split the tail count DVE || ACT.
                    # DVE: fused is_le+accum on the first DSPL columns
                    # against posR; ACT: Sign/Square on the rest against
                    # negR. Combined into asum = 2*#{c > R}.
                    DSPL = 2600
                    negr2_ps = pp.tile([P, 1], F32, tag="pst")
                    nc.tensor.matmul(out=negr2_ps[:],
                                     lhsT=kt[0:BL, C_SELBN1:C_SELBN1 + P],
                                     rhs=thrb[:], start=True, stop=False)
                    nc.tensor.matmul(out=negr2_ps[:],
                                     lhsT=kt[:, C_PL32:C_PL32 + P],
                                     rhs=Scol, start=False, stop=True)
                    nc.scalar.copy(out=negr[:, col], in_=negr2_ps[:])
                    cnt128 = sp.tile([P, 1], F32)
                    dcs = slice(blk * CH, blk * CH + DSPL)
                    nc.vector.tensor_scalar(
                        M[:, dcs], C[:, dcs], negr_ps[:], None,
                        op0=AT.is_le, op1=AT.add, accum_out=cnt128[:],
                    )
                    acs = slice(blk * CH + DSPL, (blk + 1) * CH)
                    a1b = sp.tile([P, 1], F32)
                    a2b = sp.tile([P, 1], F32)
                    nc.scalar.activation(
                        out=M[:, acs], in_=C[:, acs], func=AF.Sign,
                        bias=negr[:, col], scale=1.0, accum_out=a1b[:])
                    nc.scalar.activation(
                        out=TG[:, acs], in_=M[:, acs], func=AF.Square,
                        accum_out=a2b[:])
                    nc.vector.tensor_scalar(
                        asum[:, col], cnt128[:], -2.0, float(2 * DSPL),
                        op0=AT.mult, op1=AT.add,
                    )
                    nc.vector.tensor_add(out=asum[:, col], in0=asum[:, col],
                                         in1=a1b[:])
                    nc.vector.tensor_add(out=asum[:, col], in0=asum[:, col],
                                         in1=a2b[:])

            # recovered_b = V - (sum_p asum)/2
            rec_ps = pp.tile([BL, 1], F32, tag="pst")
            nc.tensor.matmul(out=rec_ps[:], lhsT=kt[:, C_G32A:C_G32A + BL],
                             rhs=asum[:, 0:1], start=True, stop=False)
            nc.tensor.matmul(out=rec_ps[:], lhsT=kt[:, C_G32B:C_G32B + BL],
                             rhs=asum[:, 1:2], start=False, stop=True)
            recov = sp.tile([BL, 1], F32)
            nc.vector.tensor_scalar(
                recov[:], rec_ps[:], -0.5, float(V), op0=AT.mult, op1=AT.add
            )
            # u ~= 1.0 edge: count == V means no cdf > thr; reference gives 0
            rmask = sp.tile([BL, 1], F32)
            nc.vector.tensor_scalar(
                rmask[:], recov[:], float(V), None, op0=AT.is_lt
            )
            nc.vector.tensor_mul(out=recov[:], in0=recov[:], in1=rmask[:])

            # ---- output assembly ----
            # tok = mlt*(d+1) + meq*(last+1) - 1
            #     = [mlt*(d+1) + meq - 1] + meq*last; PRE is hoisted by Tile
            pos9 = kt[0:BL, C_POS:C_POS + 9]
            allacc = sp.tile([BL, 1], F32)
            nc.vector.tensor_scalar(
                allacc[:], num_acc, float(K), None, op0=AT.is_equal
            )
            inv = sp.tile([BL, 1], F32)
            nc.vector.tensor_scalar(
                inv[:], allacc[:], -1.0, 1.0, op0=AT.mult, op1=AT.add
            )
            t1 = sp.tile([BL, 1], F32)
            nc.vector.tensor_mul(out=t1[:], in0=allacc[:],
                                 in1=kt[0:BL, C_AUXB:C_AUXB + 1])

            outi_f = sp.tile([BL, 16], F32)
            nc.vector.memset(outi_f[:], 0.0)
            dpad = sp.tile([BL, 9], F32)
            nc.vector.tensor_scalar_add(dpad[:], kt[0:BL, C_AUXD:C_AUXD + 9],
                                        1.0)  # draft+1 (pad col -> 1)
            mlt = sp.tile([BL, 9], F32)
            nc.vector.tensor_scalar(mlt[:], pos9, num_acc, None, op0=AT.is_lt)
            meq = sp.tile([BL, 9], F32)
            nc.vector.tensor_scalar(meq[:], pos9, num_acc, None, op0=AT.is_equal)
            pre = sp.tile([BL, 9], F32)
            nc.vector.tensor_mul(out=pre[:], in0=mlt[:], in1=dpad[:])
            nc.vector.tensor_add(out=pre[:], in0=pre[:], in1=meq[:])
            nc.vector.tensor_scalar_add(pre[:], pre[:], -1.0)
            # recov-dependent tail (short):
            last = sp.tile([BL, 1], F32)
            nc.vector.tensor_scalar(
                last[:], recov[:], inv[:], t1[:], op0=AT.mult, op1=AT.add)
            t4 = sp.tile([BL, 9], F32)
            nc.vector.tensor_scalar(t4[:], meq[:], last[:], None, op0=AT.mult)
            tok = outi_f[:, 0:9]
            nc.vector.tensor_add(out=tok, in0=pre[:], in1=t4[:])

            nc.vector.tensor_copy(out=outi_f[:, 9:10], in_=num_acc)
            nc.vector.tensor_copy(out=outi_f[:, 10:11], in_=num_acc)
            nc.vector.tensor_copy(out=outi_f[:, 11:12], in_=inv[:])
            nc.vector.tensor_copy(out=outi_f[:, 12:13], in_=allacc[:])
            nc.vector.tensor_copy(out=outi_f[:, 13:14], in_=recov[:])
            nc.vector.tensor_copy(out=outi_f[:, 14:15], in_=jv)

            outi_i = sp.tile([BL, 16], I32)
            nc.vector.tensor_copy(out=outi_i[:], in_=outi_f[:])
            ofs = sp.tile([BL, 2], F32)
            nc.vector.tensor_add(out=ofs[:, 0:1], in0=tot_sb[:, 0:1],
                                 in1=tot_sb[:, 1:2])
            nc.vector.tensor_copy(out=ofs[:, 1:2], in_=num_acc)

            nc.sync.dma_start(out=out_i[:, :], in_=outi_i[:])
            nc.sync.dma_start(out=out_f[:, :], in_=ofs[:])

    nc.compile()
    return nc


_CACHE = {}
LAST_RESULTS = None


def _get_nc():
    if "nc" not in _CACHE:
        _CACHE["nc"] = build_nc()
    return _CACHE["nc"]


def kernel(
    draft_token_ids, draft_probs, target_probs, bonus_token_ids, uniform_samples
):
    global LAST_RESULTS
    dt = np.ascontiguousarray(np.asarray(draft_token_ids, dtype=np.int32))
    dp = np.asarray(draft_probs, dtype=np.float32)
    tp = np.asarray(target_probs, dtype=np.float32)
    bt = np.ascontiguousarray(np.asarray(bonus_token_ids, dtype=np.int32))
    us = np.ascontiguousarray(np.asarray(uniform_samples, dtype=np.float32))

    in_maps = []
    for i in range(NCORES):
        sl = slice(i * BL, (i + 1) * BL)
        in_maps.append({
            "draft_probs": np.ascontiguousarray(dp[sl]),
            "target_probs": np.ascontiguousarray(tp[sl]),
            "konst": _build_konst(dt[sl], us[sl], bt[sl]),
        })

    res = run_bass_kernel_spmd(_get_nc(), in_maps, core_ids=list(range(NCORES)))
    LAST_RESULTS = res

    out = np.full((B, K + 1), PLACEHOLDER, np.int32)
    num_acc = np.zeros(B, np.int32)
    rec_counts = np.zeros(B, np.int32)
    bon_counts = np.zeros(B, np.int32)
    for i in range(NCORES):
        oi = np.asarray(res.results[i]["out_i"])
        of = np.asarray(res.results[i]["out_f"])
        sl = slice(i * BL, (i + 1) * BL)
        out[sl] = oi[:, 0:K + 1]
        num_acc[sl] = oi[:, 9]
        rec_counts[sl] = oi[:, 11]
        bon_counts[sl] = oi[:, 12]
        # degenerate fallback: total <= 0 -> reference picks argmax(target_j)
        tot = of[:, 0]
        for b in np.where(tot <= 0.0)[0]:
            g = i * BL + b
            na = int(num_acc[g])
            if na < K:
                j = min(na, K - 1)
                out[g, na] = int(np.argmax(tp[g, j]))

    return (
        out,
        num_acc.copy(),
        num_acc.copy(),
        rec_counts,
        bon_counts,
    )


# revision 22
# speedup vs baseline: 1.0146x; 1.0146x over previous
"""Trainium2 Bass kernel: fused speculative-decoding rejection sampling.

Problem: B=64 requests x K=8 draft tokens over V=128000 vocab.
Data-parallel over 8 NeuronCores (8 requests per core). Each core:
  1. indirect-gathers p=target[b,k,d], q=draft[b,k,d] (64 scalars, one per
     partition -- HW indirect DMA consumes one index per output partition),
  2. computes the accept/reject chain with small block-triangular matmuls
     -> num_accepted, first-reject position j, u_j, all in [8,1] layout,
  3. indirect-gathers ONLY the j-th rows of target/draft probs
     (2 tensors x 2 blocks x 2 halves x 1MB; row layout: 32 partitions x
     4000) -- the memory-roofline trick: 8MB/core instead of 64MB/core,
  4. residual r = max(t-d,0) via min + fused scan (state=(t+state)-min,
     chained across halves), partition-prefix via block-triangular matmul,
     recovered index = #{cdf <= u*total} counted with Sign/Square on the
     Scalar engine (exact: a1=sum sign(c-R), a2=sum sign^2 ->
     count = N - (a1+a2)/2; ties give sign 0 and cancel),
  5. assembles output tokens + counts on device.
All tiny per-core scalars (token ids, uniforms, flat gather indices, bonus)
ride along in the konst tensor as exact f32 (< 2^24). The host only shards
inputs, packs konst, unpacks the device outputs, and handles the degenerate
total<=0 fallback (argmax row) which cannot occur for normalized softmax
inputs but is matched for exactness.
"""

import numpy as np

import concourse.bacc as bacc
import concourse.mybir as mybir
from concourse.bass import IndirectOffsetOnAxis
from concourse.bass_utils import run_bass_kernel_spmd
from concourse.tile import TileContext

B, K, V = 64, 8, 128000
NCORES = 8
BL = B // NCORES          # 8 batch rows per core
BK = BL * K               # 64 (b,k) pairs per core
P = 128                   # SBUF partitions
RP = 32                   # partitions per gathered row
CH = V // RP              # 4000 row elements per partition
HCH = CH // 2             # half-chunk (per-gather free extent)
NBLK = 2                  # batch blocks (4 batches each)
BPB = BL // NBLK          # 4 batches per block
PLACEHOLDER = -1

# konst (f32 [128, KW]) column layout
C_IOTA32 = 0              # [128,1] p % 32
C_KP1 = 1                 # [64,1] m%8 + 1
C_K64 = 2                 # [64,1] m%8
C_C0 = 3                  # [64,1] 1 if m%8==0
C_ROWB = 4                # [8,1] 8*b
C_POS = 5                 # [8, 5:14] = 0..8
C_LINC = 16               # [64, 16:80]  (k//8==m//8)&(k<=m)      lhsT[k,m]
C_SH = 80                 # [64, 80:144] first-reject shift matrix lhsT[k,m]
C_GEXT = 144              # [64, 144:152] (m//8 == b)             lhsT[m,b]
C_SELB1 = 152             # [8, 152:280]  (p//32 == b%4)          lhsT[b,p]
C_SELB32 = 280            # [8, 280:408]  32*(p//32 == b%4)       lhsT[b,p]
C_PL32 = 408              # [128, 408:536] (k//32==m//32)&(k<m)   lhsT[k,m]
C_G32A = 536              # [128, 536:544] (k//32==b%4)&(b<4)     lhsT[k,b]
C_G32B = 544              # [128, 544:552] (k//32==b%4)&(b>=4)    lhsT[k,b]
C_AUXFI = 552             # [64,1] flat gather index (b*K+k)*V + d (exact f32)
C_AUXU = 553              # [64,1] uniform samples, (b,k) on partitions
C_AUXD = 554              # [8, 554:563] draft tokens padded with a zero col
C_AUXB = 563              # [8,1] bonus token ids
C_BLKM = 564              # [8, 564:566] 1 if b//4 == blk
C_SELBN1 = 566            # [8, 566:694]  -(p//32 == b%4)         lhsT[b,p]
C_NL32 = 694              # [128, 694:822] -((k//32==m//32)&(k<m)) lhsT[k,m]
KW = 822

AT = mybir.AluOpType
AF = mybir.ActivationFunctionType
F32 = mybir.dt.float32
I32 = mybir.dt.int32


def _build_konst(dloc: np.ndarray, uloc: np.ndarray, bloc: np.ndarray
                 ) -> np.ndarray:
    """dloc [BL,K] int draft ids, uloc [BL,K] f32 uniforms, bloc [BL] bonus."""
    k = np.zeros((P, KW), np.float32)
    p = np.arange(P)
    m = np.arange(BK)
    b8 = np.arange(BL)
    k[:, C_IOTA32] = 2 * (p % RP)   # half-row units
    k[:BK, C_KP1] = m % K + 1
    k[:BK, C_K64] = m % K
    k[:BK, C_C0] = (m % K == 0)
    k[:BL, C_ROWB] = b8 * K
    k[:BL, C_POS:C_POS + 9] = np.arange(K + 1)[None, :]
    k[:BK, C_LINC:C_LINC + BK] = (
        (m[:, None] // K == m[None, :] // K) & (m[:, None] <= m[None, :])
    )
    # oh_m = SH^T@alive + c0: first-reject one-hot (j = min(num_acc, K-1))
    sh = np.zeros((BK, BK), np.float32)
    for mm in range(BK):
        r = mm % K
        if r == 0:
            sh[mm, mm] = -1.0            # oh_0 = 1 - alive_0 (c0 adds the 1)
        elif r < K - 1:
            sh[mm - 1, mm] = 1.0         # oh_m = alive_{m-1} - alive_m
            sh[mm, mm] = -1.0
        else:
            sh[mm - 1, mm] = 1.0         # oh_7 = alive_6
    k[:BK, C_SH:C_SH + BK] = sh
    k[:BK, C_GEXT:C_GEXT + BL] = (m[:, None] // K == b8[None, :])
    selb = (p[None, :] // RP == (b8 % BPB)[:, None]).astype(np.float32)
    k[:BL, C_SELB1:C_SELB1 + P] = selb
    k[:BL, C_SELB32:C_SELB32 + P] = 2 * RP * selb   # half-row units
    k[:, C_PL32:C_PL32 + P] = (
        (p[:, None] // RP == p[None, :] // RP) & (p[:, None] < p[None, :])
    )
    ga = (p[:, None] // RP == (b8 % BPB)[None, :]).astype(np.float32)
    k[:, C_G32A:C_G32A + BL] = ga * (b8[None, :] // BPB == 0)
    k[:, C_G32B:C_G32B + BL] = ga * (b8[None, :] // BPB == 1)
    k[:BK, C_AUXFI] = (m * V + dloc.reshape(BK)).astype(np.float32)
    k[:BK, C_AUXU] = uloc.reshape(BK)
    k[:BL, C_AUXD:C_AUXD + K] = dloc
    k[:BL, C_AUXB] = bloc
    k[:BL, C_BLKM:C_BLKM + NBLK] = (b8[:, None] // BPB == np.arange(NBLK)[None, :])
    k[:BL, C_SELBN1:C_SELBN1 + P] = -selb
    k[:, C_NL32:C_NL32 + P] = -k[:, C_PL32:C_PL32 + P]
    return k


def build_nc():
    nc = bacc.Bacc("TRN2", target_bir_lowering=False, debug=False)

    dpr = nc.declare_dram_parameter("draft_probs", [BL, K, V], F32, isOutput=False)
    tpr = nc.declare_dram_parameter("target_probs", [BL, K, V], F32, isOutput=False)
    kon = nc.declare_dram_parameter("konst", [P, KW], F32, isOutput=False)
    out_i = nc.declare_dram_parameter("out_i", [BL, 16], I32, isOutput=True)
    out_f = nc.declare_dram_parameter("out_f", [BL, 2], F32, isOutput=True)

    # DRAM views for the indirect gathers (offset must be 0)
    tp_flat = tpr.ap().rearrange("b k (v one) -> (b k v) one", one=1)
    dp_flat = dpr.ap().rearrange("b k (v one) -> (b k v) one", one=1)
    # [BL*K*64, 2000] half-row-chunk table (coef = HCH per index step)
    tp_rows = tpr.ap().rearrange("b k (h l) -> (b k h) l", l=HCH)
    dp_rows = dpr.ap().rearrange("b k (h l) -> (b k h) l", l=HCH)

    with TileContext(nc) as tc:
        with (
            tc.tile_pool(name="sb", bufs=1) as sp,
            tc.tile_pool(name="ps", bufs=2, space="PSUM") as pp,
        ):
            kt = sp.tile([P, KW], F32)
            nc.sync.dma_start(out=kt[:], in_=kon[:, :])
            u64 = kt[0:BK, C_AUXU:C_AUXU + 1]

            # ---- p/q element gathers (one element per partition) ----
            fidx_i = sp.tile([BK, 1], I32)
            nc.vector.tensor_copy(out=fidx_i[:],
                                  in_=kt[0:BK, C_AUXFI:C_AUXFI + 1])
            p64 = sp.tile([BK, 1], F32)
            q64 = sp.tile([BK, 1], F32)
            nc.gpsimd.indirect_dma_start(
                out=p64[:], out_offset=None, in_=tp_flat,
                in_offset=IndirectOffsetOnAxis(ap=fidx_i[:], axis=0),
            )
            nc.gpsimd.indirect_dma_start(
                out=q64[:], out_offset=None, in_=dp_flat,
                in_offset=IndirectOffsetOnAxis(ap=fidx_i[:], axis=0),
            )

            # ---- accept chain: accept = u*max(q,1e-10) < p ----
            qc = sp.tile([BK, 1], F32)
            nc.vector.tensor_scalar_max(qc[:], q64[:], 1e-10)
            uq = sp.tile([BK, 1], F32)
            nc.vector.tensor_mul(out=uq[:], in0=u64, in1=qc[:])
            acc = sp.tile([BK, 1], F32)
            nc.vector.tensor_tensor(out=acc[:], in0=uq[:], in1=p64[:], op=AT.is_lt)
            ac_ps = pp.tile([BK, 1], F32, tag="pst")
            nc.tensor.matmul(out=ac_ps[:], lhsT=kt[0:BK, C_LINC:C_LINC + BK],
                             rhs=acc[:])
            rhs3 = sp.tile([BK, 3], F32)
            # alive_m = (#accepts through m == m%8+1)
            nc.vector.tensor_tensor(
                out=rhs3[:, 0:1], in0=ac_ps[:], in1=kt[0:BK, C_KP1:C_KP1 + 1],
                op=AT.is_equal,
            )
            oh_ps = pp.tile([BK, 1], F32, tag="pst")
            nc.tensor.matmul(out=oh_ps[:], lhsT=kt[0:BK, C_SH:C_SH + BK],
                             rhs=rhs3[:, 0:1])
            oh = sp.tile([BK, 1], F32)
            nc.vector.tensor_add(out=oh[:], in0=oh_ps[:],
                                 in1=kt[0:BK, C_C0:C_C0 + 1])
            nc.vector.tensor_mul(out=rhs3[:, 1:2], in0=oh[:],
                                 in1=kt[0:BK, C_K64:C_K64 + 1])
            nc.vector.tensor_mul(out=rhs3[:, 2:3], in0=oh[:], in1=u64)
            nj_ps = pp.tile([BL, 3], F32, tag="pst")
            nc.tensor.matmul(out=nj_ps[:], lhsT=kt[0:BK, C_GEXT:C_GEXT + BL],
                             rhs=rhs3[:])
            nj = sp.tile([BL, 3], F32)
            nc.vector.tensor_copy(out=nj[:], in_=nj_ps[:])
            num_acc = nj[:, 0:1]
            jv = nj[:, 1:2]
            u_j = nj[:, 2:3]

            # ---- block chunk indices: idx[p,blk] = rowidx*32 + p%32 ----
            rowidx = sp.tile([BL, 1], F32)
            nc.vector.tensor_add(out=rowidx[:], in0=jv,
                                 in1=kt[0:BL, C_ROWB:C_ROWB + 1])
            rsplit = sp.tile([BL, NBLK], F32)
            nc.vector.tensor_scalar(
                rsplit[:], kt[0:BL, C_BLKM:C_BLKM + NBLK], rowidx[:], None,
                op0=AT.mult,
            )
            idx_ps = pp.tile([P, NBLK], F32, tag="pst")
            nc.tensor.matmul(out=idx_ps[:], lhsT=kt[0:BL, C_SELB32:C_SELB32 + P],
                             rhs=rsplit[:])
            idx_i = sp.tile([P, NBLK], I32)
            nc.vector.tensor_scalar(
                idx_i[:], idx_ps[:], kt[:, C_IOTA32:C_IOTA32 + 1], None,
                op0=AT.add,
            )

            # ---- gather the selected rows, 1MB per gather ----
            TG = sp.tile([P, NBLK * CH], F32)
            DR = sp.tile([P, NBLK * CH], F32)
            M = sp.tile([P, NBLK * CH], F32)
            C = sp.tile([P, NBLK * CH], F32)
            # block 0's first half is quartered so the DVE stream starts
            # ~2.5us earlier; later pieces are bigger (fewer Q7 emissions).
            PIECES = {0: [(0, 1000), (1000, 2000), (2000, 4000)],
                      1: [(0, 2000), (2000, 4000)]}
            for blk in range(NBLK):
                for (p0, p1) in PIECES[blk]:
                    cs = slice(blk * CH + p0, blk * CH + p1)
                    nc.gpsimd.indirect_dma_start(
                        out=TG[:, cs], out_offset=None, in_=tp_rows,
                        in_offset=IndirectOffsetOnAxis(
                            ap=idx_i[:, blk:blk + 1], axis=0),
                        element_offset=p0,
                    )
                    nc.gpsimd.indirect_dma_start(
                        out=DR[:, cs], out_offset=None, in_=dp_rows,
                        in_offset=IndirectOffsetOnAxis(
                            ap=idx_i[:, blk:blk + 1], axis=0),
                        element_offset=p0,
                    )

            # ---- per block: residual cumsum, thresholds, counts ----
            asum = sp.tile([P, NBLK], F32)
            a2t = sp.tile([P, NBLK], F32)
            tot_sb = sp.tile([BL, NBLK], F32)
            negr = sp.tile([P, NBLK], F32)
            for blk in range(NBLK):
                for (p0, p1) in PIECES[blk]:
                    cs = slice(blk * CH + p0, blk * CH + p1)
                    nc.vector.tensor_tensor(
                        out=M[:, cs], in0=TG[:, cs], in1=DR[:, cs], op=AT.min)
                    init = 0.0 if p0 == 0 else C[:, blk * CH + p0 - 1:
                                                 blk * CH + p0]
                    nc.vector.tensor_tensor_scan(
                        out=C[:, cs], data0=TG[:, cs], data1=M[:, cs],
                        initial=init, op0=AT.add, op1=AT.subtract,
                    )
                # matmuls read the scan's last column (the row totals)
                Scol = C[:, (blk + 1) * CH - 1:(blk + 1) * CH]
                # per-block totals (other block's batches read as zero)
                totb_ps = pp.tile([BL, 1], F32, tag="pst")
                GXX = C_G32A if blk == 0 else C_G32B
                nc.tensor.matmul(out=totb_ps[:], lhsT=kt[:, GXX:GXX + BL],
                                 rhs=Scol)
                nc.scalar.copy(out=tot_sb[:, blk:blk + 1], in_=totb_ps[:])
                # thr = u_j * total (Scalar engine; keeps DVE streaming)
                thrb = sp.tile([BL, 1], F32, tag="thrn")
                nc.scalar.activation(out=thrb[:], in_=totb_ps[:],
                                     func=AF.Copy, bias=0.0, scale=u_j)
                col = slice(blk, blk + 1)
                negr_ps = pp.tile([P, 1], F32, tag="pst")
                if blk == 0:
                    # negR[p] = P_pref - thr (bias for the Sign count)
                    nc.tensor.matmul(out=negr_ps[:],
                                     lhsT=kt[0:BL, C_SELBN1:C_SELBN1 + P],
                                     rhs=thrb[:], start=True, stop=False)
                    nc.tensor.matmul(out=negr_ps[:],
                                     lhsT=kt[:, C_PL32:C_PL32 + P],
                                     rhs=Scol, start=False, stop=True)
                else:
                    # posR[p] = thr - P_pref (threshold for the DVE count)
                    nc.tensor.matmul(out=negr_ps[:],
                                     lhsT=kt[0:BL, C_SELB1:C_SELB1 + P],
                                     rhs=thrb[:], start=True, stop=False)
                    nc.tensor.matmul(out=negr_ps[:],
                                     lhsT=kt[:, C_NL32:C_NL32 + P],
                                     rhs=Scol, start=False, stop=True)
                if blk == 0:
                    nc.scalar.copy(out=negr[:, col], in_=negr_ps[:])
                if blk == 0:
                    # exact count on the Scalar engine (overlaps block-1 DVE):
                    # a1+a2 = 2*#{c > R}; ties give sign 0 and cancel
                    for h in range(2):
                        cs = slice(blk * CH + h * HCH,
                                   blk * CH + (h + 1) * HCH)
                        nc.scalar.activation(
                            out=M[:, cs], in_=C[:, cs], func=AF.Sign,
                            bias=negr[:, col], scale=1.0,
                            accum_out=(asum[:, col] if h == 0 else a2t[:, col]),
                        )
                    nc.vector.tensor_add(out=asum[:, col], in0=asum[:, col],
                                         in1=a2t[:, col])
                    for h in range(2):
                        cs = slice(blk * CH + h * HCH,
                                   blk * CH + (h + 1) * HCH)
                        nc.scalar.activation(
                            out=TG[:, cs], in_=M[:, cs], func=AF.Square,
                            accum_out=a2t[:, col],
                        )
                        nc.vector.tensor_add(out=asum[:, col],
                                             in0=asum[:, col],
                                             in1=a2t[:, col])
                else:
                    # last block: split the tail count DVE || ACT.
                    # DVE: fused is_le+accum on the first DSPL columns
                    # against posR; ACT: Sign/Square on the rest against
                    # negR. Combined into asum = 2*#{c > R}.
                    DSPL = 2600
                    negr2_ps = pp.tile([P, 1], F32, tag="pst")
                    nc.tensor.matmul(out=negr2_ps[:],
                                     lhsT=kt[0:BL, C_SELBN1:C_SELBN1 + P],
                                     rhs=thrb[:], start=True, stop=False)
                    nc.tensor.matmul(out=negr2_ps[:],
                                     lhsT=kt[:, C_PL32:C_PL32 + P],
                                     rhs=Scol, start=False, stop=True)
                    nc.scalar.copy(out=negr[:, col], in_=negr2_ps[:])
                    cnt128 = sp.tile([P, 1], F32)
                    dcs = slice(blk * CH, blk * CH + DSPL)
                    nc.vector.tensor_scalar(
                        M[:, dcs], C[:, dcs], negr_ps[:], None,
                        op0=AT.is_le, op1=AT.add, accum_out=cnt128[:],
                    )
                    acs = slice(blk * CH + DSPL, (blk + 1) * CH)
                    a1b = sp.tile([P, 1], F32)
                    a2b = sp.tile([P, 1], F32)
                    nc.scalar.activation(
                        out=M[:, acs], in_=C[:, acs], func=AF.Sign,
                        bias=negr[:, col], scale=1.0, accum_out=a1b[:])
                    nc.scalar.activation(
                        out=TG[:, acs], in_=M[:, acs], func=AF.Square,
                        accum_out=a2b[:])
                    nc.vector.tensor_scalar(
                        asum[:, col], cnt128[:], -2.0, float(2 * DSPL),
                        op0=AT.mult, op1=AT.add,
                    )
                    nc.vector.tensor_add(out=asum[:, col], in0=asum[:, col],
                                         in1=a1b[:])
                    nc.vector.tensor_add(out=asum[:, col], in0=asum[:, col],
                                         in1=a2b[:])

            # recovered_b = V - (sum_p asum)/2
            rec_ps = pp.tile([BL, 1], F32, tag="pst")
            nc.tensor.matmul(out=rec_ps[:], lhsT=kt[:, C_G32A:C_G32A + BL],
                             rhs=asum[:, 0:1], start=True, stop=False)
            nc.tensor.matmul(out=rec_ps[:], lhsT=kt[:, C_G32B:C_G32B + BL],
                             rhs=asum[:, 1:2], start=False, stop=True)
            recov = sp.tile([BL, 1], F32)
            nc.vector.tensor_scalar(
                recov[:], rec_ps[:], -0.5, float(V), op0=AT.mult, op1=AT.add
            )
            # u ~= 1.0 edge: count == V means no cdf > thr; reference gives 0
            rmask = sp.tile([BL, 1], F32)
            nc.vector.tensor_scalar(
                rmask[:], recov[:], float(V), None, op0=AT.is_lt
            )
            nc.vector.tensor_mul(out=recov[:], in0=recov[:], in1=rmask[:])

            # ---- output assembly ----
            # tok = mlt*(d+1) + meq*(last+1) - 1
            #     = [mlt*(d+1) + meq - 1] + meq*last; PRE is hoisted by Tile
            pos9 = kt[0:BL, C_POS:C_POS + 9]
            allacc = sp.tile([BL, 1], F32)
            nc.vector.tensor_scalar(
                allacc[:], num_acc, float(K), None, op0=AT.is_equal
            )
            inv = sp.tile([BL, 1], F32)
            nc.vector.tensor_scalar(
                inv[:], allacc[:], -1.0, 1.0, op0=AT.mult, op1=AT.add
            )
            t1 = sp.tile([BL, 1], F32)
            nc.vector.tensor_mul(out=t1[:], in0=allacc[:],
                                 in1=kt[0:BL, C_AUXB:C_AUXB + 1])

            outi_f = sp.tile([BL, 16], F32)
            nc.vector.memset(outi_f[:], 0.0)
            dpad = sp.tile([BL, 9], F32)
            nc.vector.tensor_scalar_add(dpad[:], kt[0:BL, C_AUXD:C_AUXD + 9],
                                        1.0)  # draft+1 (pad col -> 1)
            mlt = sp.tile([BL, 9], F32)
            nc.vector.tensor_scalar(mlt[:], pos9, num_acc, None, op0=AT.is_lt)
            meq = sp.tile([BL, 9], F32)
            nc.vector.tensor_scalar(meq[:], pos9, num_acc, None, op0=AT.is_equal)
            pre = sp.tile([BL, 9], F32)
            nc.vector.tensor_mul(out=pre[:], in0=mlt[:], in1=dpad[:])
            nc.vector.tensor_add(out=pre[:], in0=pre[:], in1=meq[:])
            nc.vector.tensor_scalar_add(pre[:], pre[:], -1.0)
            # recov-dependent tail (short):
            last = sp.tile([BL, 1], F32)
            nc.vector.tensor_scalar(
                last[:], recov[:], inv[:], t1[:], op0=AT.mult, op1=AT.add)
            t4 = sp.tile([BL, 9], F32)
            nc.vector.tensor_scalar(t4[:], meq[:], last[:], None, op0=AT.mult)
            tok = outi_f[:, 0:9]
            nc.vector.tensor_add(out=tok, in0=pre[:], in1=t4[:])

            nc.vector.tensor_copy(out=outi_f[:, 9:10], in_=num_acc)
            nc.vector.tensor_copy(out=outi_f[:, 10:11], in_=num_acc)
            nc.vector.tensor_copy(out=outi_f[:, 11:12], in_=inv[:])
            nc.vector.tensor_copy(out=outi_f[:, 12:13], in_=allacc[:])
            nc.vector.tensor_copy(out=outi_f[:, 13:14], in_=recov[:])
            nc.vector.tensor_copy(out=outi_f[:, 14:15], in_=jv)

            outi_i = sp.tile([BL, 16], I32)
            nc.vector.tensor_copy(out=outi_i[:], in_=outi_f[:])
            ofs = sp.tile([BL, 2], F32)
            nc.vector.tensor_add(out=ofs[:, 0:1], in0=tot_sb[:, 0:1],
                                 in1=tot_sb[:, 1:2])
            nc.vector.tensor_copy(out=ofs[:, 1:2], in_=num_acc)

            nc.sync.dma_start(out=out_i[:, :], in_=outi_i[:])
            nc.sync.dma_start(out=out_f[:, :], in_=ofs[:])

    nc.compile()
    return nc


_CACHE = {}
LAST_RESULTS = None


def _get_nc():
    if "nc" not in _CACHE:
        _CACHE["nc"] = build_nc()
    return _CACHE["nc"]


def kernel(
    draft_token_ids, draft_probs, target_probs, bonus_token_ids, uniform_samples
):
    global LAST_RESULTS
    dt = np.ascontiguousarray(np.asarray(draft_token_ids, dtype=np.int32))
    dp = np.asarray(draft_probs, dtype=np.float32)
    tp = np.asarray(target_probs, dtype=np.float32)
    bt = np.ascontiguousarray(np.asarray(bonus_token_ids, dtype=np.int32))
    us = np.ascontiguousarray(np.asarray(uniform_samples, dtype=np.float32))

    in_maps = []
    for i in range(NCORES):
        sl = slice(i * BL, (i + 1) * BL)
        in_maps.append({
            "draft_probs": np.ascontiguousarray(dp[sl]),
            "target_probs": np.ascontiguousarray(tp[sl]),
            "konst": _build_konst(dt[sl], us[sl], bt[sl]),
        })

    res = run_bass_kernel_spmd(_get_nc(), in_maps, core_ids=list(range(NCORES)))
    LAST_RESULTS = res

    out = np.full((B, K + 1), PLACEHOLDER, np.int32)
    num_acc = np.zeros(B, np.int32)
    rec_counts = np.zeros(B, np.int32)
    bon_counts = np.zeros(B, np.int32)
    for i in range(NCORES):
        oi = np.asarray(res.results[i]["out_i"])
        of = np.asarray(res.results[i]["out_f"])
        sl = slice(i * BL, (i + 1) * BL)
        out[sl] = oi[:, 0:K + 1]
        num_acc[sl] = oi[:, 9]
        rec_counts[sl] = oi[:, 11]
        bon_counts[sl] = oi[:, 12]
        # degenerate fallback: total <= 0 -> reference picks argmax(target_j)
        tot = of[:, 0]
        for b in np.where(tot <= 0.0)[0]:
            g = i * BL + b
            na = int(num_acc[g])
            if na < K:
                j = min(na, K - 1)
                out[g, na] = int(np.argmax(tp[g, j]))

    return (
        out,
        num_acc.copy(),
        num_acc.copy(),
        rec_counts,
        bon_counts,
    )
